# revision 3
# baseline (speedup 1.0000x reference)
"""BaseAttentionPooling Trainium2 kernel.

reference:
    h = tanh(x @ W1 + b1)            # [N, H]
    logits = (h @ W2 + b2)[:, 0]     # [N]
    per-graph softmax over sorted `batch`, pooled = seg_sum(x * w)  # [G, D]

Strategy (data-parallel over graphs, 8 cores, SPMD-identical program):
  - 512 graphs/core, split into 8 blocks of 64 graphs; nodes padded to
    `cpb` chunks of 128 per block (cpb uniform across cores/blocks).
  - Host ships TWO copies of x (pure layout/precision prep):
      * node-major bf16 for the pooling matmuls (accuracy-critical),
      * d-major fp8(e4m3) with DoubleRow k-tile interleave for the MLP
        (logits tolerate fp8; simulated end-to-end rel err 7.8e-3).
    This removes every transpose-mode LDWEIGHTS from the hot loop.
  - b2 dropped (cancels in softmax); exp without max-subtraction is safe
    because |logits| <= ||W2||_1 + |b2| is small.
  - Per chunk on PE: hT = W1.T @ xT in ONE fp8 DoubleRow matmul (K=256),
    logits via stationary-h matmul (1 col), pooledT[d, g-block] and
    den[g-block] accumulate in PSUM with normal (pipelined) weight loads.
  - oh[i, g] = (iota[g] == rel_gid[i]) * e[i] built on DVE per chunk
    ([128, 64] only — 64-graph blocks halve both DVE and pooled-MM cost).
  - Epilogue: PE-transpose pooledT -> [g, d], scale by 1/max(den, tiny).
"""

import os
import sys

import numpy as np

for _p in ("/opt/trn_rl_repo",):
    if _p not in sys.path and os.path.isdir(_p):
        sys.path.insert(0, _p)

import ml_dtypes

import concourse.bass as bass
import concourse.tile as tile
from concourse import bacc, mybir
from concourse import bass_utils

N, D, H, G = 500000, 256, 128, 4096
NCORES = 8
GPC = G // NCORES          # graphs per core = 512
BLKG = 64                  # graphs per block
NBLK = GPC // BLKG         # blocks per core = 8
P = 128                    # partition / chunk size

BF16 = mybir.dt.bfloat16
F32 = mybir.dt.float32
F8 = mybir.dt.float8e4
NP_BF16 = ml_dtypes.bfloat16
NP_F8 = ml_dtypes.float8_e4m3fn

LAST_RESULT = None  # test.py reads exec_time_ns / profile from here


# ---------------------------------------------------------------- host plan

def make_plan(batch):
    """Compute the uniform chunk layout from the sorted graph ids."""
    batch = np.asarray(batch)
    seg = np.searchsorted(batch, np.arange(G + 1), side="left")  # [G+1]
    counts = np.zeros((NCORES, NBLK), dtype=np.int64)
    for c in range(NCORES):
        for b in range(NBLK):
            g0 = c * GPC + b * BLKG
            counts[c, b] = seg[g0 + BLKG] - seg[g0]
    cpb = int(np.ceil(counts.max() / P))
    cpb = ((cpb + 7) // 8) * 8     # whole number of 8-chunk groups
    ch = NBLK * cpb                # chunks per core
    return seg, cpb, ch


def build_inputs(x, batch, W1, b1, W2, seg, cpb, ch):
    """Build the 8 per-core input maps (layout/precision prep only)."""
    x = np.asarray(x)
    batch = np.asarray(batch)
    n_g8 = ch // 8

    w1_f = np.asarray(W1, dtype=np.float32)          # [256, 128]
    # DoubleRow k-tile interleave: w1dr[p, t*128+m] = W1[t*128+p, m]
    w1dr = np.ascontiguousarray(
        w1_f.reshape(2, P, H).transpose(1, 0, 2).reshape(P, 2 * H)
    ).astype(NP_F8)
    b1_f = np.asarray(b1, dtype=np.float32).reshape(H, 1)
    w2_bf = np.asarray(W2, dtype=np.float32).astype(NP_BF16).reshape(H, 1)
    iota = np.broadcast_to(
        np.arange(BLKG, dtype=np.float32), (P, BLKG)
    ).astype(NP_BF16)                                 # iota[p, f] = f
    idf = np.eye(P, dtype=np.float32)

    in_maps = []
    for c in range(NCORES):
        xpad = np.zeros((ch * P, D), dtype=np.float32)
        rel = np.full(ch * P, -1.0, dtype=np.float32)
        for b in range(NBLK):
            g0 = c * GPC + b * BLKG
            s0, s1 = int(seg[g0]), int(seg[g0 + BLKG])
            n = s1 - s0
            r0 = b * cpb * P
            xpad[r0 : r0 + n] = x[s0:s1]
            rel[r0 : r0 + n] = (batch[s0:s1] - g0).astype(np.float32)
        # node-major bf16, tiled so each 8-chunk group is one contiguous
        # [128, 2048] DRAM block (4KB per partition row)
        xs_t = np.ascontiguousarray(
            xpad.astype(NP_BF16)
            .reshape(n_g8, 8, P, D)
            .transpose(0, 2, 1, 3)
            .reshape(n_g8 * P, 8 * D)
        )
        # d-major fp8 with DoubleRow interleave:
        # xt_t[g8*128+p, (t*8+j)*128 + i] = xpad[(g8*8+j)*128 + i, t*128+p]
        xt_t = np.ascontiguousarray(
            xpad.astype(NP_F8)
            .reshape(n_g8, 8, P, 2, P)       # [g8, j, i, t, p]
            .transpose(0, 4, 3, 1, 2)        # [g8, p, t, j, i]
            .reshape(n_g8 * P, 2 * 8 * P)
        )
        blr = np.ascontiguousarray(rel.reshape(ch, P).T)  # [128, ch] f32
        in_maps.append(
            {
                "xs": xs_t,
                "xt": xt_t,
                "blr": blr,
                "w1": w1dr,
                "b1": b1_f,
                "w2": w2_bf,
                "iota": iota,
                "idf": idf,
            }
        )
    return in_maps


# ------------------------------------------------------------- bass program

def build_bass(ch, cpb):
    """Build the SPMD-uniform per-core program."""
    nc = bacc.Bacc(
        "TRN2",
        target_bir_lowering=False,
        debug=False,
        num_devices=NCORES,
    )
    n_g8 = ch // 8
    xs = nc.dram_tensor("xs", [n_g8 * P, 8 * D], BF16, kind="ExternalInput").ap()
    xt = nc.dram_tensor("xt", [n_g8 * P, 16 * P], F8, kind="ExternalInput").ap()
    blr = nc.dram_tensor("blr", [P, ch], F32, kind="ExternalInput").ap()
    w1 = nc.dram_tensor("w1", [P, 2 * H], F8, kind="ExternalInput").ap()
    b1 = nc.dram_tensor("b1", [H, 1], F32, kind="ExternalInput").ap()
    w2 = nc.dram_tensor("w2", [H, 1], BF16, kind="ExternalInput").ap()
    iota = nc.dram_tensor("iota", [P, BLKG], BF16, kind="ExternalInput").ap()
    idf = nc.dram_tensor("idf", [P, P], F32, kind="ExternalInput").ap()
    out = nc.dram_tensor("out", [GPC, D], F32, kind="ExternalOutput").ap()

    DR = mybir.MatmulPerfMode.DoubleRow

    with tile.TileContext(nc) as tc:
        with (
            tc.tile_pool(name="consts", bufs=1) as cpool,
            tc.tile_pool(name="xb", bufs=3) as xbpool,
            tc.tile_pool(name="xtp", bufs=3) as xtpool,
            tc.tile_pool(name="hsb", bufs=3) as hsbpool,
            tc.tile_pool(name="e8", bufs=2) as epool,
            tc.tile_pool(name="oh", bufs=20) as ohpool,
            tc.tile_pool(name="outsb", bufs=2) as outpool,
            tc.tile_pool(name="acc", bufs=1, space="PSUM") as accpool,
            tc.tile_pool(name="hps", bufs=2, space="PSUM") as hpool,
            tc.tile_pool(name="lg", bufs=2, space="PSUM") as lgpool,
            tc.tile_pool(name="ep", bufs=1, space="PSUM") as eppool,
        ):
            # ---- constants into SBUF
            w1_sb = cpool.tile([P, 2 * H], F8, tag="w1")
            b1_sb = cpool.tile([H, 1], F32, tag="b1")
            w2_sb = cpool.tile([H, 1], BF16, tag="w2")
            io_sb = cpool.tile([P, BLKG], BF16, tag="iota")
            id_sb = cpool.tile([P, P], F32, tag="idf")
            ones_sb = cpool.tile([P, 1], BF16, tag="ones")
            blr_sb = cpool.tile([P, ch], F32, tag="blr")
            nc.sync.dma_start(w1_sb[:], w1[:])
            nc.sync.dma_start(b1_sb[:], b1[:])
            nc.sync.dma_start(w2_sb[:], w2[:])
            nc.sync.dma_start(io_sb[:], iota[:])
            nc.sync.dma_start(id_sb[:], idf[:])
            nc.sync.dma_start(blr_sb[:], blr[:])
            nc.vector.memset(ones_sb[:], 1.0)
            w1_ap = w1_sb[:].rearrange("p (t m) -> p t m", t=2)

            # ---- persistent accumulators (PSUM)
            pooled_lo = accpool.tile([P, NBLK * BLKG], F32, tag="plo")
            pooled_hi = accpool.tile([P, NBLK * BLKG], F32, tag="phi")
            den = accpool.tile([BLKG, NBLK], F32, tag="den")

            def flush_pooled(items):
                # one group late so PE has W1/logits work while DVE builds oh
                for oh, xb, j, c in items:
                    b = c // cpb
                    first = c == b * cpb
                    last = c == (b + 1) * cpb - 1
                    col = b * BLKG
                    nc.tensor.matmul(
                        pooled_lo[:, col : col + BLKG],
                        xb[:, j * D : j * D + P],
                        oh[:],
                        start=first,
                        stop=last,
                    )
                    nc.tensor.matmul(
                        pooled_hi[:, col : col + BLKG],
                        xb[:, j * D + P : (j + 1) * D],
                        oh[:],
                        start=first,
                        stop=last,
                    )
                    nc.tensor.matmul(
                        den[:, b : b + 1],
                        oh[:],
                        ones_sb[:],
                        start=first,
                        stop=last,
                    )

            pending = []
            for g8 in range(n_g8):
                xb = xbpool.tile([P, 8 * D], BF16)
                nc.sync.dma_start(xb[:], xs[g8 * P : (g8 + 1) * P, :])
                xtp = xtpool.tile([P, 16 * P], F8)
                nc.sync.dma_start(xtp[:], xt[g8 * P : (g8 + 1) * P, :])
                xt_ap = xtp[:].rearrange("p (t n) -> p t n", t=2)  # [128,2,1024]
                lg = lgpool.tile([P, 8], F32)
                for half in range(2):
                    hps = hpool.tile([P, 4 * P], F32)
                    nc.tensor.matmul(
                        hps[:],
                        w1_ap,
                        xt_ap[:, :, half * 512 : (half + 1) * 512],
                        start=True,
                        stop=True,
                        perf_mode=DR,
                    )
                    hsb = hsbpool.tile([P, 4 * P], BF16)
                    nc.scalar.activation(
                        hsb[:], hps[:],
                        mybir.ActivationFunctionType.Tanh, bias=b1_sb[:],
                    )
                    for k in range(4):
                        j = half * 4 + k
                        nc.tensor.matmul(
                            lg[:, j : j + 1],
                            hsb[:, k * P : (k + 1) * P],
                            w2_sb[:],
                            start=True,
                            stop=True,
                        )
                    if half == 0 and pending:
                        flush_pooled(pending)
                        pending = []
                e8 = epool.tile([P, 8], F32)
                nc.scalar.activation(
                    e8[:], lg[:], mybir.ActivationFunctionType.Exp
                )
                for j in range(8):
                    c = g8 * 8 + j
                    oh = ohpool.tile([P, BLKG], BF16)
                    # oh[i, g] = (iota[g] == rel_gid[i]) * e[i]
                    nc.vector.tensor_scalar(
                        oh[:],
                        io_sb[:],
                        blr_sb[:, c : c + 1],
                        e8[:, j : j + 1],
                        mybir.AluOpType.is_equal,
                        mybir.AluOpType.mult,
                    )
                    pending.append((oh, xb, j, c))
            flush_pooled(pending)
            pending = []

            # ---- epilogue: out[g] = pooledT.T[g] / max(denom[g], tiny)
            plo_sb = outpool.tile([P, NBLK * BLKG], F32, tag="plo_sb")
            phi_sb = outpool.tile([P, NBLK * BLKG], F32, tag="phi_sb")
            nc.vector.tensor_copy(plo_sb[:], pooled_lo[:])
            nc.vector.tensor_copy(phi_sb[:], pooled_hi[:])
            dmax = outpool.tile([BLKG, NBLK], F32, tag="dmax")
            rec = outpool.tile([BLKG, NBLK], F32, tag="rec")
            nc.vector.tensor_scalar_max(dmax[:], den[:], 1e-30)
            nc.vector.reciprocal(rec[:], dmax[:])
            for b in range(NBLK):
                tp = eppool.tile([BLKG, 2 * P], F32)
                nc.tensor.transpose(
                    tp[:, 0:P], plo_sb[:, b * BLKG : (b + 1) * BLKG], id_sb[:]
                )
                nc.tensor.transpose(
                    tp[:, P : 2 * P], phi_sb[:, b * BLKG : (b + 1) * BLKG], id_sb[:]
                )
                osb = outpool.tile([BLKG, D], F32, tag="osb")
                nc.scalar.mul(osb[:], tp[:], rec[:, b : b + 1])
                nc.sync.dma_start(out[b * BLKG : (b + 1) * BLKG, :], osb[:])

    nc.compile()
    return nc


# ----------------------------------------------------------------- kernel()

def kernel(**inputs):
    global LAST_RESULT
    x = np.asarray(inputs["x"])
    batch = np.asarray(inputs["batch"])
    W1 = np.asarray(inputs["W1"])
    b1 = np.asarray(inputs["b1"])
    W2 = np.asarray(inputs["W2"])
    # b2 cancels in the softmax; unused.

    seg, cpb, ch = make_plan(batch)
    in_maps = build_inputs(x, batch, W1, b1, W2, seg, cpb, ch)
    nc = build_bass(ch, cpb)
    res = bass_utils.run_bass_kernel_spmd(
        nc, in_maps, list(range(NCORES))
    )
    LAST_RESULT = res
    out = np.concatenate(
        [np.asarray(res.results[c]["out"]) for c in range(NCORES)], axis=0
    )
    return out.astype(np.float32)


# revision 6
# speedup vs baseline: 1.3105x; 1.3105x over previous
"""BaseAttentionPooling Trainium2 kernel.

reference:
    h = tanh(x @ W1 + b1)            # [N, H]
    logits = (h @ W2 + b2)[:, 0]     # [N]
    per-graph softmax over sorted `batch`, pooled = seg_sum(x * w)  # [G, D]

Strategy (data-parallel over graphs, 8 cores, SPMD-identical program):
  - 512 graphs/core, split into 8 blocks of 64 graphs; nodes padded to
    `cpb` chunks of 128 per block (cpb uniform across cores/blocks).
  - Host ships TWO copies of x (pure layout/precision prep):
      * node-major bf16 for the pooling matmuls (accuracy-critical),
      * d-major fp8(e4m3) with DoubleRow k-tile interleave for the MLP
        (logits tolerate fp8; simulated end-to-end rel err ~1.1e-2).
    This removes every transpose-mode LDWEIGHTS from the hot loop.
  - b2 dropped (cancels in softmax); exp without max-subtraction is safe
    because |logits| <= ||W2||_1 + |b2| is small.
  - PE per chunk: hT via ONE fp8 DoubleRow matmul (K=256, batched over
    4 chunks), logits via stationary-h matmul (1 col), pooled via
    stationary-oh matmul with the long 256-col moving x hiding the
    next weight loads; den shares the oh stationary.  Pooled lands
    [g, d] directly -> no epilogue transposes.
  - oh[i, g] = (iota[g] == rel_gid[i]) * e[i] built on DVE per chunk
    ([128, 64] — 64-graph blocks to halve DVE cost).
  - PSUM: two 64-graph blocks pack into each [128, 256] f32 tile
    (partition halves) so all 8 block accumulators fit in 2 banks.
"""

import os
import sys

import numpy as np

for _p in ("/opt/trn_rl_repo",):
    if _p not in sys.path and os.path.isdir(_p):
        sys.path.insert(0, _p)

import ml_dtypes

import concourse.bass as bass
import concourse.tile as tile
from concourse import bacc, mybir
from concourse import bass_utils

N, D, H, G = 500000, 256, 128, 4096
NCORES = 8
GPC = G // NCORES          # graphs per core = 512
BLKG = 64                  # graphs per block
NBLK = GPC // BLKG         # blocks per core = 8
P = 128                    # partition / chunk size

BF16 = mybir.dt.bfloat16
F32 = mybir.dt.float32
F8 = mybir.dt.float8e4
NP_BF16 = ml_dtypes.bfloat16
NP_F8 = ml_dtypes.float8_e4m3fn

LAST_RESULT = None  # test.py reads exec_time_ns / profile from here


# ---------------------------------------------------------------- host plan

def make_plan(batch):
    """Compute the uniform chunk layout from the sorted graph ids."""
    batch = np.asarray(batch)
    seg = np.searchsorted(batch, np.arange(G + 1), side="left")  # [G+1]
    counts = np.zeros((NCORES, NBLK), dtype=np.int64)
    for c in range(NCORES):
        for b in range(NBLK):
            g0 = c * GPC + b * BLKG
            counts[c, b] = seg[g0 + BLKG] - seg[g0]
    cpb = int(np.ceil(counts.max() / P))
    cpb = ((cpb + 7) // 8) * 8     # whole number of 8-chunk groups
    ch = NBLK * cpb                # chunks per core
    return seg, cpb, ch


def build_inputs(x, batch, W1, b1, W2, seg, cpb, ch):
    """Build the 8 per-core input maps (layout/precision prep only)."""
    x = np.asarray(x)
    batch = np.asarray(batch)
    n_g8 = ch // 8

    w1_f = np.asarray(W1, dtype=np.float32)          # [256, 128]
    # DoubleRow k-tile interleave: w1dr[p, t*128+m] = W1[t*128+p, m]
    w1dr = np.ascontiguousarray(
        w1_f.reshape(2, P, H).transpose(1, 0, 2).reshape(P, 2 * H)
    ).astype(NP_F8)
    b1_f = np.asarray(b1, dtype=np.float32).reshape(H, 1)
    w2_bf = np.asarray(W2, dtype=np.float32).astype(NP_BF16).reshape(H, 1)
    iota = np.broadcast_to(
        np.arange(BLKG, dtype=np.float32), (P, BLKG)
    ).astype(NP_BF16)                                 # iota[p, f] = f

    in_maps = []
    for c in range(NCORES):
        xpad = np.zeros((ch * P, D), dtype=np.float32)
        rel = np.full(ch * P, -1.0, dtype=np.float32)
        for b in range(NBLK):
            g0 = c * GPC + b * BLKG
            s0, s1 = int(seg[g0]), int(seg[g0 + BLKG])
            n = s1 - s0
            r0 = b * cpb * P
            xpad[r0 : r0 + n] = x[s0:s1]
            rel[r0 : r0 + n] = (batch[s0:s1] - g0).astype(np.float32)
        # node-major bf16, tiled so each 8-chunk group is one contiguous
        # [128, 2048] DRAM block (4KB per partition row)
        xs_t = np.ascontiguousarray(
            xpad.astype(NP_BF16)
            .reshape(n_g8, 8, P, D)
            .transpose(0, 2, 1, 3)
            .reshape(n_g8 * P, 8 * D)
        )
        # d-major fp8 with DoubleRow interleave:
        # xt_t[g8*128+p, (t*8+j)*128 + i] = xpad[(g8*8+j)*128 + i, t*128+p]
        xt_t = np.ascontiguousarray(
            xpad.astype(NP_F8)
            .reshape(n_g8, 8, P, 2, P)       # [g8, j, i, t, p]
            .transpose(0, 4, 3, 1, 2)        # [g8, p, t, j, i]
            .reshape(n_g8 * P, 2 * 8 * P)
        )
        blr = np.ascontiguousarray(rel.reshape(ch, P).T)  # [128, ch] f32
        in_maps.append(
            {
                "xs": xs_t,
                "xt": xt_t,
                "blr": blr,
                "w1": w1dr,
                "b1": b1_f,
                "w2": w2_bf,
                "iota": iota,
            }
        )
    return in_maps


# ------------------------------------------------------------- bass program

def build_bass(ch, cpb):
    """Build the SPMD-uniform per-core program."""
    nc = bacc.Bacc(
        "TRN2",
        target_bir_lowering=False,
        debug=False,
        num_devices=NCORES,
    )
    n_g8 = ch // 8
    xs = nc.dram_tensor("xs", [n_g8 * P, 8 * D], BF16, kind="ExternalInput").ap()
    xt = nc.dram_tensor("xt", [n_g8 * P, 16 * P], F8, kind="ExternalInput").ap()
    blr = nc.dram_tensor("blr", [P, ch], F32, kind="ExternalInput").ap()
    w1 = nc.dram_tensor("w1", [P, 2 * H], F8, kind="ExternalInput").ap()
    b1 = nc.dram_tensor("b1", [H, 1], F32, kind="ExternalInput").ap()
    w2 = nc.dram_tensor("w2", [H, 1], BF16, kind="ExternalInput").ap()
    iota = nc.dram_tensor("iota", [P, BLKG], BF16, kind="ExternalInput").ap()
    out = nc.dram_tensor("out", [GPC, D], F32, kind="ExternalOutput").ap()

    DR = mybir.MatmulPerfMode.DoubleRow

    with tile.TileContext(nc) as tc:
        with (
            tc.tile_pool(name="consts", bufs=1) as cpool,
            tc.tile_pool(name="xb", bufs=3) as xbpool,
            tc.tile_pool(name="xtp", bufs=3) as xtpool,
            tc.tile_pool(name="hsb", bufs=3) as hsbpool,
            tc.tile_pool(name="e8", bufs=2) as epool,
            tc.tile_pool(name="oh", bufs=20) as ohpool,
            tc.tile_pool(name="outsb", bufs=2) as outpool,
            tc.tile_pool(name="acc", bufs=1, space="PSUM") as accpool,
            tc.tile_pool(name="hps", bufs=2, space="PSUM") as hpool,
            tc.tile_pool(name="lg", bufs=2, space="PSUM") as lgpool,
        ):
            # ---- constants into SBUF
            w1_sb = cpool.tile([P, 2 * H], F8, tag="w1")
            b1_sb = cpool.tile([H, 1], F32, tag="b1")
            w2_sb = cpool.tile([H, 1], BF16, tag="w2")
            io_sb = cpool.tile([P, BLKG], BF16, tag="iota")
            ones_sb = cpool.tile([P, 1], BF16, tag="ones")
            blr_sb = cpool.tile([P, ch], F32, tag="blr")
            nc.sync.dma_start(w1_sb[:], w1[:])
            nc.sync.dma_start(b1_sb[:], b1[:])
            nc.sync.dma_start(w2_sb[:], w2[:])
            nc.sync.dma_start(io_sb[:], iota[:])
            nc.sync.dma_start(blr_sb[:], blr[:])
            nc.vector.memset(ones_sb[:], 1.0)
            w1_ap = w1_sb[:].rearrange("p (t m) -> p t m", t=2)

            # ---- persistent accumulators (PSUM)
            # four 64-graph blocks pack into each [128, 512] f32 tile
            # (one PSUM bank): partition halves x column halves
            ppA = accpool.tile([P, 2 * D], F32, tag="ppA")
            ppB = accpool.tile([P, 2 * D], F32, tag="ppB")
            den = accpool.tile([BLKG, NBLK], F32, tag="den")

            def pooled_out(b):
                t = ppA if b < 4 else ppB
                r0 = (b % 2) * BLKG
                c0 = ((b % 4) // 2) * D
                return t[r0 : r0 + BLKG, c0 : c0 + D]

            def flush_one(item, lg_thunk=None):
                # pooled[g, :] += oh.T @ x ; den[g] += oh.T @ 1.
                # The 256-col pooled matmul hides the following weight
                # loads (den's oh reload, the next logits' hsb load).
                oh, xb, j, c = item
                b = c // cpb
                first = c == b * cpb
                last = c == (b + 1) * cpb - 1
                nc.tensor.matmul(
                    pooled_out(b),
                    oh[:],
                    xb[:, j * D : (j + 1) * D],
                    start=first,
                    stop=last,
                )
                nc.tensor.matmul(
                    den[:, b : b + 1],
                    oh[:],
                    ones_sb[:],
                    start=first,
                    stop=last,
                )
                if lg_thunk is not None:
                    lg_thunk()

            pending = []
            for g8 in range(n_g8):
                xb = xbpool.tile([P, 8 * D], BF16)
                nc.sync.dma_start(xb[:], xs[g8 * P : (g8 + 1) * P, :])
                xtp = xtpool.tile([P, 16 * P], F8)
                nc.sync.dma_start(xtp[:], xt[g8 * P : (g8 + 1) * P, :])
                xt_ap = xtp[:].rearrange("p (t n) -> p t n", t=2)  # [128,2,1024]
                lg = lgpool.tile([P, 8], F32)
                for half in range(2):
                    hps = hpool.tile([P, 4 * P], F32)
                    nc.tensor.matmul(
                        hps[:],
                        w1_ap,
                        xt_ap[:, :, half * 512 : (half + 1) * 512],
                        start=True,
                        stop=True,
                        perf_mode=DR,
                    )
                    hsb = hsbpool.tile([P, 4 * P], BF16)
                    nc.scalar.activation(
                        hsb[:], hps[:],
                        mybir.ActivationFunctionType.Tanh, bias=b1_sb[:],
                    )
                    for k in range(4):
                        j = half * 4 + k

                        def lg_mm(j=j, k=k, hsb=hsb, lg=lg):
                            nc.tensor.matmul(
                                lg[:, j : j + 1],
                                hsb[:, k * P : (k + 1) * P],
                                w2_sb[:],
                                start=True,
                                stop=True,
                            )

                        if pending:
                            flush_one(pending.pop(0), lg_mm)
                        else:
                            lg_mm()
                e8 = epool.tile([P, 8], F32)
                nc.scalar.activation(
                    e8[:], lg[:], mybir.ActivationFunctionType.Exp
                )
                for j in range(8):
                    c = g8 * 8 + j
                    oh = ohpool.tile([P, BLKG], BF16)
                    # oh[i, g] = (iota[g] == rel_gid[i]) * e[i]
                    nc.vector.tensor_scalar(
                        oh[:],
                        io_sb[:],
                        blr_sb[:, c : c + 1],
                        e8[:, j : j + 1],
                        mybir.AluOpType.is_equal,
                        mybir.AluOpType.mult,
                    )
                    pending.append((oh, xb, j, c))
            for item in pending:
                flush_one(item)
            pending = []

            # ---- epilogue: out[g] = pooled[g] / max(denom[g], tiny)
            dmax = outpool.tile([BLKG, NBLK], F32, tag="dmax")
            rec = outpool.tile([BLKG, NBLK], F32, tag="rec")
            nc.vector.tensor_scalar_max(dmax[:], den[:], 1e-30)
            nc.vector.reciprocal(rec[:], dmax[:])
            for b in range(NBLK):
                osb = outpool.tile([BLKG, D], F32, tag="osb")
                nc.scalar.mul(osb[:], pooled_out(b), rec[:, b : b + 1])
                nc.sync.dma_start(out[b * BLKG : (b + 1) * BLKG, :], osb[:])

    nc.compile()
    return nc


# ----------------------------------------------------------------- kernel()

def kernel(**inputs):
    global LAST_RESULT
    x = np.asarray(inputs["x"])
    batch = np.asarray(inputs["batch"])
    W1 = np.asarray(inputs["W1"])
    b1 = np.asarray(inputs["b1"])
    W2 = np.asarray(inputs["W2"])
    # b2 cancels in the softmax; unused.

    seg, cpb, ch = make_plan(batch)
    in_maps = build_inputs(x, batch, W1, b1, W2, seg, cpb, ch)
    nc = build_bass(ch, cpb)
    res = bass_utils.run_bass_kernel_spmd(
        nc, in_maps, list(range(NCORES))
    )
    LAST_RESULT = res
    out = np.concatenate(
        [np.asarray(res.results[c]["out"]) for c in range(NCORES)], axis=0
    )
    return out.astype(np.float32)
